# revision 19
# baseline (speedup 1.0000x reference)
"""AttentionBlock (GroupNorm -> 1x1 qkv -> 4-head attention -> 1x1 proj -> residual)
on 8 trn2 NeuronCores, data-parallel over the batch dim (B=8, one element/core).

v2: fp8e4m3 + DoubleRow for all contraction>=256 matmuls (qkv, v, PV, proj);
ST stays bf16 (contraction=128/head). Softmax rowsum is computed on the PE
with column-tiled accumulating ones-matmuls (2 head-strips concurrent) instead
of a DVE add tree. exp is shifted by -SHIFT so probabilities fit fp8 range
(softmax is invariant). Residual comes from the bf16 input copy; the f32
input is never loaded.

Layouts per core (channel-major, spatial N=1024):
  xb[t]   [128, 1024] bf16   input tiles (also the residual)
  hs8     [128, 4*1024] fp8  groupnorm out, k-subtile blocks for DoubleRow
  wt8     [128, 4*1536] fp8  qkv weights  w[c+128k, o] at (c, k*1536+o)
  pwt8    [128, 4*512]  fp8  proj weights w[h*128+c, o] at (c, h*512+o)
  qks[8]  [128, 1024] bf16   q (0-3) / k (4-7) head tiles
  v8      [128, 8*512]  fp8  v spatial-major, jt blocks
  pt8[h]  [128, 8*1024] fp8  exp(scale*ST - SHIFT), jt blocks
  attn8   [128, 4*1024] fp8  normalized attention out, head blocks
"""

import numpy as np

B, C, H, W = 8, 512, 32, 32
N = H * W  # 1024
NUM_HEADS = 4
HEAD_DIM = C // NUM_HEADS  # 128
NUM_GROUPS = 32
GROUP_CH = C // NUM_GROUPS  # 16
EPS = 1e-5
NT = C // 128  # 4 channel tiles
NO_QK = 8  # q,k output tiles (1024 channels)
SCALE = 1.0 / float(np.sqrt(HEAD_DIM))
SHIFT = 4.0  # logit shift so exp() fits fp8e4m3
N_CORES = 8


def build_bass():
    import concourse.bacc as bacc
    import concourse.tile as tile
    from concourse import mybir

    f32 = mybir.dt.float32
    bf16 = mybir.dt.bfloat16
    f8 = mybir.dt.float8e4
    DR = mybir.MatmulPerfMode.DoubleRow
    Act = mybir.ActivationFunctionType
    Alu = mybir.AluOpType
    Ax = mybir.AxisListType

    nc = bacc.Bacc("TRN2", target_bir_lowering=False, debug=False,
                   num_devices=N_CORES)

    d_xb = nc.declare_dram_parameter("xb", [C, N], bf16, isOutput=False)
    d_wt = nc.declare_dram_parameter("qkv_wt8", [128, 4 * 3 * C], f8,
                                     isOutput=False)
    d_pwt = nc.declare_dram_parameter("proj_wt8", [128, 4 * C], f8,
                                      isOutput=False)
    d_cv = nc.declare_dram_parameter("cvec", [128, 28], f32, isOutput=False)
    d_selT = nc.declare_dram_parameter("selT", [8, 128], f32, isOutput=False)
    d_ones8 = nc.declare_dram_parameter("ones8", [128, 32], f8, isOutput=False)
    d_out = nc.declare_dram_parameter("out", [C, N], f32, isOutput=True)

    with tile.TileContext(nc) as tc:
        with (
            tc.tile_pool(name="persist", bufs=1) as pp,
            tc.tile_pool(name="outp", bufs=2) as p_out,
            tc.tile_pool(name="small", bufs=1) as ps,
            tc.tile_pool(name="psum", bufs=2, space="PSUM") as pm,
        ):
            # ---- constants + loads. xb first on the two HWDGE queues;
            # weights + small constants on the gpsimd SWDGE queue.
            warm = ps.tile([128, 512], bf16, tag="warm", name="warm")
            nc.vector.memset(warm[:], 0.5)
            epsv = ps.tile([8, 1], f32, tag="epsv", name="epsv")
            nc.vector.memset(epsv[:], EPS)
            shv = ps.tile([128, 1], f32, tag="shv", name="shv")
            nc.vector.memset(shv[:], -SHIFT)
            cvec = ps.tile([128, 28], f32, tag="cvec", name="cvec")
            nc.scalar.dma_start(cvec[:], d_cv[:, :])
            gam, bet, bqk, beff = (cvec[:, 0:4], cvec[:, 4:8],
                                   cvec[:, 8:16], cvec[:, 16:20])
            sel = cvec[:, 20:28]

            xbs = []
            for t in range(NT):
                xb_t = pp.tile([128, N], bf16, tag=f"xb{t}", name=f"xb{t}")
                eng = nc.scalar if t % 2 else nc.sync
                eng.dma_start(xb_t[:], d_xb[t * 128:(t + 1) * 128, :])
                xbs.append(xb_t)
            selT = ps.tile([8, 128], f32, tag="selT", name="selT")
            nc.gpsimd.dma_start(selT[:], d_selT[:, :])
            wt8 = pp.tile([128, 4 * 3 * C], f8, tag="wt8", name="wt8")
            nc.gpsimd.dma_start(wt8[:, 0:6 * C], d_wt[:, 0:6 * C])
            nc.gpsimd.dma_start(wt8[:, 6 * C:12 * C], d_wt[:, 6 * C:12 * C])
            ones8 = ps.tile([128, 32], f8, tag="ones8", name="ones8")
            nc.gpsimd.dma_start(ones8[:], d_ones8[:, :])
            pwt8 = pp.tile([128, 4 * C], f8, tag="pwt8", name="pwt8")
            nc.gpsimd.dma_start(pwt8[:], d_pwt[:, :])

            hs8 = pp.tile([128, NT * N], f8, tag="hs8", name="hs8")
            v8 = pp.tile([128, NO_QK * C], f8, tag="v8", name="v8")
            attn8 = pp.tile([128, NUM_HEADS * N], f8, tag="attn8",
                            name="attn8")
            pt8 = []
            for h in range(NUM_HEADS):
                pt8_h = pp.tile([128, NO_QK * N], f8, tag=f"pt8_{h}",
                                name=f"pt8_{h}")
                pt8.append(pt8_h)
            rbs, rrs = [], []
            for h in range(NUM_HEADS):
                rbs.append(pp.tile([128, N], f32, tag=f"rb{h}", name=f"rb{h}"))
                rrs.append(pp.tile([1, N], f32, tag=f"rr{h}", name=f"rr{h}"))

            # 3D views for DoubleRow operands: [part, k-subtile, inner]
            wt8_3 = wt8[:].rearrange("p (k o) -> p k o", o=3 * C)
            hs8_3 = hs8[:].rearrange("p (k i) -> p k i", i=N)
            v8_3 = v8[:].rearrange("p (k o) -> p k o", o=C)
            pwt8_3 = pwt8[:].rearrange("p (k o) -> p k o", o=C)
            attn8_3 = attn8[:].rearrange("p (k i) -> p k i", i=N)
            pt8_3 = [pt8[h][:].rearrange("p (k i) -> p k i", i=N)
                     for h in range(NUM_HEADS)]
            ones8_3 = ones8[:].rearrange("p (k m) -> p k m", m=16)[:, :, 0:1]

            # PE warm-up: junk matmul chain (never read) keeps the HAM
            # clock-gate open while inputs stream in.
            junk = pm.tile([128, N], f32, tag="acc", name="junk")

            def junk_mm(n, first=False, last=False):
                for j in range(n):
                    nc.tensor.matmul(junk[0:128, 0:512], warm[:, 0:128],
                                     warm[:, 0:512],
                                     start=(first and j == 0),
                                     stop=(last and j == n - 1),
                                     skip_group_check=True)

            junk_mm(8, first=True)

            # ---- group norm, per-tile (groups never cross 128-ch tiles)
            for t in range(NT):
                st_t = ps.tile([128, 2], f32, tag=f"st{t}", name=f"st{t}")
                nc.vector.reduce_sum(st_t[:, 0:1], xbs[t][:], axis=Ax.X)
                nc.scalar.activation(hs8[:, t * N:(t + 1) * N], xbs[t][:],
                                     Act.Square, accum_out=st_t[:, 1:2])
                psg = pm.tile([128, N], f32, tag="ps", name=f"psg{t}")
                nc.tensor.matmul(psg[0:8, 0:2], sel, st_t[:, 0:2],
                                 start=True, stop=True)
                inv_n = 1.0 / float(GROUP_CH * N)
                msr = ps.tile([8, 4], f32, tag=f"msr{t}", name=f"msr{t}")
                nc.scalar.mul(msr[:, 0:1], psg[0:8, 0:1], inv_n)
                nc.scalar.square(msr[:, 3:4], msr[:, 0:1])
                nc.vector.scalar_tensor_tensor(msr[:, 2:3], psg[0:8, 1:2],
                                               inv_n, msr[:, 3:4],
                                               op0=Alu.mult, op1=Alu.subtract)
                # rstd = exp(-0.5*ln(var+eps)): stays in the ln/exp table
                # set (no switch before the attention exps)
                nc.scalar.activation(msr[:, 3:4], msr[:, 2:3], Act.Ln,
                                     bias=epsv[:, 0:1])
                nc.scalar.activation(msr[:, 1:2], msr[:, 3:4], Act.Exp,
                                     scale=-0.5)
                pse = pm.tile([128, N], f32, tag="ps", name=f"pse{t}")
                nc.tensor.matmul(pse[:, 0:2], selT[:], msr[:, 0:2],
                                 start=True, stop=True)
                ab_t = ps.tile([128, 3], f32, tag=f"ab{t}", name=f"ab{t}")
                nc.vector.tensor_mul(ab_t[:, 0:1], gam[:, t:t + 1], pse[:, 1:2])
                nc.vector.tensor_mul(ab_t[:, 2:3], pse[:, 0:1], ab_t[:, 0:1])
                nc.vector.tensor_sub(ab_t[:, 1:2], bet[:, t:t + 1], ab_t[:, 2:3])
                if t % 2 == 0:
                    nc.scalar.activation(hs8[:, t * N:(t + 1) * N], xbs[t][:],
                                         Act.Identity,
                                         bias=ab_t[:, 1:2], scale=ab_t[:, 0:1])
                else:
                    nc.vector.tensor_scalar(hs8[:, t * N:(t + 1) * N],
                                            xbs[t][:],
                                            ab_t[:, 0:1], ab_t[:, 1:2],
                                            op0=Alu.mult, op1=Alu.add)
                junk_mm(4, last=(t == NT - 1))

            qks = [None] * NO_QK
            ppvs = [None] * NUM_HEADS
            prss = [None] * NUM_HEADS

            def emit_qk_half(ot, half):
                # fp8 DoubleRow: contract 512 channels as 2 pairs of 128
                if emit_qk_half.pq[ot] is None:
                    emit_qk_half.pq[ot] = pm.tile([128, N], f32, tag="acc",
                                                  name=f"pq{ot}")
                pq = emit_qk_half.pq[ot]
                for kp in range(2):
                    nc.tensor.matmul(
                        pq[:, half * 512:(half + 1) * 512],
                        wt8_3[:, 2 * kp:2 * kp + 2, ot * 128:(ot + 1) * 128],
                        hs8_3[:, 2 * kp:2 * kp + 2, half * 512:(half + 1) * 512],
                        start=(kp == 0), stop=(kp == 1), perf_mode=DR)
                if half == 1:
                    qk_t = pp.tile([128, N], bf16, tag=f"qk{ot}",
                                   name=f"qk{ot}")
                    nc.vector.tensor_scalar_add(qk_t[:], pq[:],
                                                bqk[:, ot:ot + 1])
                    qks[ot] = qk_t
            emit_qk_half.pq = [None] * NO_QK

            def emit_qk(ot):
                emit_qk_half(ot, 0)
                emit_qk_half(ot, 1)

            def emit_v(nt):
                pv_ = pm.tile([128, N], f32, tag="acc", name=f"pvv{nt}")
                for kp in range(2):
                    nc.tensor.matmul(
                        pv_[:, 0:512],
                        hs8_3[:, 2 * kp:2 * kp + 2, nt * 128:(nt + 1) * 128],
                        wt8_3[:, 2 * kp:2 * kp + 2, 2 * C:3 * C],
                        start=(kp == 0), stop=(kp == 1), perf_mode=DR)
                nc.vector.tensor_copy(v8[:, nt * C:(nt + 1) * C],
                                      pv_[:, 0:512])

            def emit_st(h, jt):
                qT = qks[h]
                kT = qks[NUM_HEADS + h]
                pst = pm.tile([128, N], f32, tag="ps", name=f"pst{h}_{jt}")
                for half in range(2):
                    nc.tensor.matmul(
                        pst[:, half * 512:(half + 1) * 512],
                        kT[:, jt * 128:(jt + 1) * 128],
                        qT[:, half * 512:(half + 1) * 512],
                        start=True, stop=True)
                nc.scalar.activation(pt8[h][:, jt * N:(jt + 1) * N], pst[:],
                                     Act.Exp, scale=SCALE, bias=shv[:, 0:1])

            def emit_rowsum_unit(h, half, p4):
                # DoubleRow ones-matmul: rowsum of a jt-pair of exp tiles
                if prss[h] is None:
                    prss[h] = pm.tile([128, N], f32, tag="acc",
                                      name=f"prs{h}")
                nc.tensor.matmul(
                    prss[h][0:1, half * 512:(half + 1) * 512],
                    ones8_3,
                    pt8_3[h][:, 2 * p4:2 * p4 + 2,
                             half * 512:(half + 1) * 512],
                    start=(p4 == 0), stop=(p4 == 3), perf_mode=DR)

            def emit_recip_bcast(h):
                nc.vector.reciprocal_approx_fast(rrs[h][:],
                                                 prss[h][0:1, :])
                nc.gpsimd.partition_broadcast(rbs[h][:], rrs[h][:])

            def emit_pv_unit(h, half, p4):
                if ppvs[h] is None:
                    ppvs[h] = pm.tile([128, N], f32, tag="acc", name=f"ppv{h}")
                nc.tensor.matmul(
                    ppvs[h][:, half * 512:(half + 1) * 512],
                    v8_3[:, 2 * p4:2 * p4 + 2, h * 128:(h + 1) * 128],
                    pt8_3[h][:, 2 * p4:2 * p4 + 2,
                             half * 512:(half + 1) * 512],
                    start=(p4 == 0), stop=(p4 == 3), perf_mode=DR)

            def emit_pv(h):
                for half in range(2):
                    for p4 in range(4):
                        emit_pv_unit(h, half, p4)

            def emit_norm(h, half=None):
                halves = range(2) if half is None else [half]
                for hf in halves:
                    sl = slice(h * N + hf * 512, h * N + (hf + 1) * 512)
                    psl = slice(hf * 512, (hf + 1) * 512)
                    nc.vector.tensor_mul(attn8[:, sl], ppvs[h][:, psl],
                                         rbs[h][:, psl])

            # ---- attention pipeline, interleaved so PE never waits on ACT
            emit_qk(0)
            emit_qk(4)
            emit_st(0, 0)
            emit_qk_half(1, 0); emit_st(0, 1)
            emit_qk_half(1, 1); emit_st(0, 2)
            emit_qk_half(5, 0); emit_st(0, 3)
            emit_qk_half(5, 1); emit_st(0, 4)
            emit_qk_half(2, 0); emit_st(0, 5)
            emit_qk_half(2, 1); emit_st(0, 6)
            emit_qk_half(6, 0); emit_st(0, 7)
            emit_qk_half(6, 1)
            emit_st(1, 0)
            emit_qk_half(3, 0); emit_st(1, 1)
            emit_qk_half(3, 1); emit_st(1, 2)
            emit_qk_half(7, 0); emit_st(1, 3)
            emit_qk_half(7, 1); emit_st(1, 4)
            emit_v(0); emit_st(1, 5)
            emit_v(1); emit_st(1, 6)
            emit_v(2); emit_st(1, 7)
            emit_v(3)
            emit_st(2, 0)
            emit_v(4); emit_st(2, 1)
            emit_v(5); emit_st(2, 2)
            emit_v(6); emit_st(2, 3)
            emit_v(7); emit_st(2, 4)
            emit_rowsum_unit(0, 0, 0); emit_st(2, 5)
            emit_rowsum_unit(0, 0, 1); emit_st(2, 6)
            emit_rowsum_unit(0, 0, 2); emit_st(2, 7)
            emit_rowsum_unit(0, 0, 3)
            emit_st(3, 0)
            emit_rowsum_unit(0, 1, 0); emit_rowsum_unit(0, 1, 1)
            emit_st(3, 1)
            emit_rowsum_unit(0, 1, 2); emit_rowsum_unit(0, 1, 3)
            emit_recip_bcast(0)
            emit_st(3, 2)
            emit_rowsum_unit(1, 0, 0); emit_rowsum_unit(1, 0, 1)
            emit_st(3, 3)
            emit_rowsum_unit(1, 0, 2); emit_rowsum_unit(1, 0, 3)
            emit_st(3, 4)
            emit_rowsum_unit(1, 1, 0); emit_rowsum_unit(1, 1, 1)
            emit_st(3, 5)
            emit_rowsum_unit(1, 1, 2); emit_rowsum_unit(1, 1, 3)
            emit_recip_bcast(1)
            emit_st(3, 6)
            emit_pv_unit(0, 0, 0); emit_pv_unit(0, 0, 1)
            emit_st(3, 7)
            emit_pv_unit(0, 0, 2); emit_pv_unit(0, 0, 3)
            emit_norm(0, 0)
            emit_pv_unit(0, 1, 0); emit_pv_unit(0, 1, 1)
            emit_pv_unit(0, 1, 2); emit_pv_unit(0, 1, 3)
            emit_norm(0, 1)
            for hf in range(2):
                for p4 in range(4):
                    emit_pv_unit(1, hf, p4)
                emit_norm(1, hf)
            for half in range(2):
                for p4 in range(4):
                    emit_rowsum_unit(2, half, p4)
            emit_recip_bcast(2)
            for hf in range(2):
                for p4 in range(4):
                    emit_pv_unit(2, hf, p4)
                emit_norm(2, hf)
            for half in range(2):
                for p4 in range(4):
                    emit_rowsum_unit(3, half, p4)
            emit_recip_bcast(3)
            for hf in range(2):
                for p4 in range(4):
                    emit_pv_unit(3, hf, p4)
                emit_norm(3, hf)

            # ---- proj + bias + residual (fp8 DoubleRow over head pairs);
            # bias+residual and the store are pipelined per half tile
            for ot in range(NT):
                ppr = pm.tile([128, N], f32, tag="acc", name=f"ppr{ot}")
                o_t = p_out.tile([128, N], f32, tag="out", name=f"o{ot}")
                for half in range(2):
                    sl = slice(half * 512, (half + 1) * 512)
                    for hp in range(2):
                        nc.tensor.matmul(
                            ppr[:, sl],
                            pwt8_3[:, 2 * hp:2 * hp + 2,
                                   ot * 128:(ot + 1) * 128],
                            attn8_3[:, 2 * hp:2 * hp + 2, sl],
                            start=(hp == 0), stop=(hp == 1), perf_mode=DR)
                    nc.vector.scalar_tensor_tensor(o_t[:, sl], ppr[:, sl],
                                                   beff[:, ot:ot + 1],
                                                   xbs[ot][:, sl],
                                                   op0=Alu.add, op1=Alu.add)
                eng = nc.scalar if ot % 2 == 1 else nc.sync
                eng.dma_start(d_out[ot * 128:(ot + 1) * 128, :], o_t[:])

    nc.compile()
    return nc


def make_in_maps(x, norm_w, norm_b, qkv_w, qkv_b, proj_w, proj_b):
    import ml_dtypes
    f8 = ml_dtypes.float8_e4m3

    x = np.asarray(x, dtype=np.float32)
    qkv_w = np.asarray(qkv_w, dtype=np.float32)
    qkv_b = np.asarray(qkv_b, dtype=np.float32)
    proj_w = np.asarray(proj_w, dtype=np.float32)
    proj_b = np.asarray(proj_b, dtype=np.float32)

    # DoubleRow k-subtile layout: wt8[c, k*1536+o] = qkv_w[o, c+128k]
    wt8 = np.ascontiguousarray(
        qkv_w.T.reshape(4, 128, 3 * C).transpose(1, 0, 2).reshape(128, -1)
    ).astype(f8)
    pwt8 = np.ascontiguousarray(
        proj_w.T.reshape(4, 128, C).transpose(1, 0, 2).reshape(128, -1)
    ).astype(f8)

    b_eff = (proj_b + proj_w @ qkv_b[2 * C:3 * C]).astype(np.float32)
    bias_qk = np.ascontiguousarray(qkv_b[:2 * C])

    p = np.arange(128)
    sel = (p[:, None] // GROUP_CH == np.arange(8)[None, :]).astype(np.float32)
    selT = np.ascontiguousarray(sel.T)

    xs = x.reshape(B, C, N)
    cvec = np.zeros((128, 28), np.float32)
    cvec[:, 0:4] = np.asarray(norm_w, np.float32).reshape(4, 128).T
    cvec[:, 4:8] = np.asarray(norm_b, np.float32).reshape(4, 128).T
    cvec[:, 8:16] = bias_qk.reshape(8, 128).T
    cvec[:, 16:20] = b_eff.reshape(4, 128).T
    cvec[:, 20:28] = sel
    common = {
        "qkv_wt8": wt8, "proj_wt8": pwt8, "cvec": cvec, "selT": selT,
        "ones8": np.ones((128, 32), np.float32).astype(f8),
    }
    return [dict(common,
                 xb=np.ascontiguousarray(xs[i]).astype(ml_dtypes.bfloat16))
            for i in range(B)]


def run(inputs, trace=False, tmpdir=None):
    from concourse.bass_utils import run_bass_kernel_spmd
    nc = build_bass()
    in_maps = make_in_maps(**inputs)
    res = run_bass_kernel_spmd(nc, in_maps, core_ids=list(range(N_CORES)),
                               trace=trace, tmpdir=tmpdir)
    out = np.stack([res.results[i]["out"] for i in range(N_CORES)])
    return out.reshape(B, C, H, W).astype(np.float32), res


def kernel(**inputs):
    out, _ = run(inputs, trace=False)
    return out


# revision 22
# speedup vs baseline: 1.2005x; 1.2005x over previous
"""AttentionBlock (GroupNorm -> 1x1 qkv -> 4-head attention -> 1x1 proj -> residual)
on 8 trn2 NeuronCores, data-parallel over the batch dim (B=8, one element/core).

v2: fp8e4m3 + DoubleRow for all contraction>=256 matmuls (qkv, v, PV, proj);
ST stays bf16 (contraction=128/head). Softmax rowsum is computed on the PE
with column-tiled accumulating ones-matmuls (2 head-strips concurrent) instead
of a DVE add tree. exp is shifted by -SHIFT so probabilities fit fp8 range
(softmax is invariant). Residual comes from the bf16 input copy; the f32
input is never loaded.

Layouts per core (channel-major, spatial N=1024):
  xb[t]   [128, 1024] bf16   input tiles (also the residual)
  hs8     [128, 4*1024] fp8  groupnorm out, k-subtile blocks for DoubleRow
  wt8     [128, 4*1536] fp8  qkv weights  w[c+128k, o] at (c, k*1536+o)
  pwt8    [128, 4*512]  fp8  proj weights w[h*128+c, o] at (c, h*512+o)
  qks[8]  [128, 1024] bf16   q (0-3) / k (4-7) head tiles
  v8      [128, 8*512]  fp8  v spatial-major, jt blocks
  pt8[h]  [128, 8*1024] fp8  exp(scale*ST - SHIFT), jt blocks
  attn8   [128, 4*1024] fp8  normalized attention out, head blocks
"""

import numpy as np

B, C, H, W = 8, 512, 32, 32
N = H * W  # 1024
NUM_HEADS = 4
HEAD_DIM = C // NUM_HEADS  # 128
NUM_GROUPS = 32
GROUP_CH = C // NUM_GROUPS  # 16
EPS = 1e-5
NT = C // 128  # 4 channel tiles
NO_QK = 8  # q,k output tiles (1024 channels)
SCALE = 1.0 / float(np.sqrt(HEAD_DIM))
SHIFT = 4.0  # logit shift so exp() fits fp8e4m3
N_CORES = 8


def build_bass():
    import concourse.bacc as bacc
    import concourse.tile as tile
    from concourse import mybir

    f32 = mybir.dt.float32
    bf16 = mybir.dt.bfloat16
    f8 = mybir.dt.float8e4
    DR = mybir.MatmulPerfMode.DoubleRow
    Act = mybir.ActivationFunctionType
    Alu = mybir.AluOpType
    Ax = mybir.AxisListType

    nc = bacc.Bacc("TRN2", target_bir_lowering=False, debug=False,
                   num_devices=N_CORES)

    d_xb = nc.declare_dram_parameter("xb", [C, N], bf16, isOutput=False)
    d_wt = nc.declare_dram_parameter("qkv_wt8", [128, 4 * 3 * C], f8,
                                     isOutput=False)
    d_pwt = nc.declare_dram_parameter("proj_wt8", [128, 4 * C], f8,
                                      isOutput=False)
    d_cv = nc.declare_dram_parameter("cvec", [128, 28], f32, isOutput=False)
    d_selT = nc.declare_dram_parameter("selT", [8, 128], f32, isOutput=False)
    d_ones8 = nc.declare_dram_parameter("ones8", [128, 32], f8, isOutput=False)
    d_out = nc.declare_dram_parameter("out", [C, N], f32, isOutput=True)

    with tile.TileContext(nc) as tc:
        with (
            tc.tile_pool(name="persist", bufs=1) as pp,
            tc.tile_pool(name="outp", bufs=2) as p_out,
            tc.tile_pool(name="small", bufs=1) as ps,
            tc.tile_pool(name="psum", bufs=2, space="PSUM") as pm,
        ):
            # ---- constants + loads. xb first on the two HWDGE queues;
            # weights + small constants on the gpsimd SWDGE queue.
            warm = ps.tile([128, 512], bf16, tag="warm", name="warm")
            nc.vector.memset(warm[:], 0.5)
            epsv = ps.tile([8, 1], f32, tag="epsv", name="epsv")
            nc.vector.memset(epsv[:], EPS)
            shv = ps.tile([128, 1], f32, tag="shv", name="shv")
            nc.vector.memset(shv[:], -SHIFT)
            cvec = ps.tile([128, 28], f32, tag="cvec", name="cvec")
            nc.scalar.dma_start(cvec[:], d_cv[:, :])
            gam, bet, bqk, beff = (cvec[:, 0:4], cvec[:, 4:8],
                                   cvec[:, 8:16], cvec[:, 16:20])
            sel = cvec[:, 20:28]

            xbs = []
            for t in range(NT):
                xb_t = pp.tile([128, N], bf16, tag=f"xb{t}", name=f"xb{t}")
                eng = nc.scalar if t % 2 else nc.sync
                eng.dma_start(xb_t[:], d_xb[t * 128:(t + 1) * 128, :])
                xbs.append(xb_t)
            selT = ps.tile([8, 128], f32, tag="selT", name="selT")
            nc.gpsimd.dma_start(selT[:], d_selT[:, :])
            wt8 = pp.tile([128, 4 * 3 * C], f8, tag="wt8", name="wt8")
            nc.gpsimd.dma_start(wt8[:, 0:6 * C], d_wt[:, 0:6 * C])
            nc.gpsimd.dma_start(wt8[:, 6 * C:12 * C], d_wt[:, 6 * C:12 * C])
            ones8 = ps.tile([128, 32], f8, tag="ones8", name="ones8")
            nc.gpsimd.dma_start(ones8[:], d_ones8[:, :])
            pwt8 = pp.tile([128, 4 * C], f8, tag="pwt8", name="pwt8")
            nc.gpsimd.dma_start(pwt8[:], d_pwt[:, :])

            hs8 = pp.tile([128, NT * N], f8, tag="hs8", name="hs8")
            v8 = pp.tile([128, NO_QK * C], f8, tag="v8", name="v8")
            attn8 = pp.tile([128, NUM_HEADS * N], f8, tag="attn8",
                            name="attn8")
            pt8 = []
            for h in range(NUM_HEADS):
                pt8_h = pp.tile([128, NO_QK * N], f8, tag=f"pt8_{h}",
                                name=f"pt8_{h}")
                pt8.append(pt8_h)
            rbs, rrs = [], []
            for h in range(NUM_HEADS):
                rbs.append(pp.tile([128, N], f32, tag=f"rb{h}", name=f"rb{h}"))
                rrs.append(pp.tile([1, N], f32, tag=f"rr{h}", name=f"rr{h}"))

            # 3D views for DoubleRow operands: [part, k-subtile, inner]
            wt8_3 = wt8[:].rearrange("p (k o) -> p k o", o=3 * C)
            hs8_3 = hs8[:].rearrange("p (k i) -> p k i", i=N)
            v8_3 = v8[:].rearrange("p (k o) -> p k o", o=C)
            pwt8_3 = pwt8[:].rearrange("p (k o) -> p k o", o=C)
            attn8_3 = attn8[:].rearrange("p (k i) -> p k i", i=N)
            pt8_3 = [pt8[h][:].rearrange("p (k i) -> p k i", i=N)
                     for h in range(NUM_HEADS)]
            ones8_3 = ones8[:].rearrange("p (k m) -> p k m", m=16)[:, :, 0:1]

            sqscr = ps.tile([128, N], bf16, tag="sqscr", name="sqscr")

            # PE warm-up: junk matmul chain (never read) keeps the HAM
            # clock-gate open while inputs stream in.
            junk = pm.tile([128, N], f32, tag="acc", name="junk")

            def junk_mm(n, first=False, last=False):
                for j in range(n):
                    nc.tensor.matmul(junk[0:128, 0:512], warm[:, 0:128],
                                     warm[:, 0:512],
                                     start=(first and j == 0),
                                     stop=(last and j == n - 1),
                                     skip_group_check=True)

            junk_mm(8, first=True)

            # ---- group norm. All Squares are emitted before any Ln/Exp so
            # ScalarE switches table sets exactly twice, not per tile.
            sts = []
            for t in range(NT):
                st_t = ps.tile([128, 2], f32, tag=f"st{t}", name=f"st{t}")
                nc.vector.reduce_sum(st_t[:, 0:1], xbs[t][:], axis=Ax.X)
                nc.scalar.activation(sqscr[:], xbs[t][:], Act.Square,
                                     accum_out=st_t[:, 1:2])
                sts.append(st_t)
            for t in range(NT):
                st_t = sts[t]
                psg = pm.tile([128, N], f32, tag="ps", name=f"psg{t}")
                nc.tensor.matmul(psg[0:8, 0:2], sel, st_t[:, 0:2],
                                 start=True, stop=True)
                inv_n = 1.0 / float(GROUP_CH * N)
                msr = ps.tile([8, 4], f32, tag=f"msr{t}", name=f"msr{t}")
                nc.scalar.mul(msr[:, 0:1], psg[0:8, 0:1], inv_n)
                nc.scalar.square(msr[:, 3:4], msr[:, 0:1])
                nc.vector.scalar_tensor_tensor(msr[:, 2:3], psg[0:8, 1:2],
                                               inv_n, msr[:, 3:4],
                                               op0=Alu.mult, op1=Alu.subtract)
                # rstd = exp(-0.5*ln(var+eps)): stays in the ln/exp table
                # set (no switch before the attention exps)
                nc.scalar.activation(msr[:, 3:4], msr[:, 2:3], Act.Ln,
                                     bias=epsv[:, 0:1])
                nc.scalar.activation(msr[:, 1:2], msr[:, 3:4], Act.Exp,
                                     scale=-0.5)
                pse = pm.tile([128, N], f32, tag="ps", name=f"pse{t}")
                nc.tensor.matmul(pse[:, 0:2], selT[:], msr[:, 0:2],
                                 start=True, stop=True)
                ab_t = ps.tile([128, 3], f32, tag=f"ab{t}", name=f"ab{t}")
                nc.vector.tensor_mul(ab_t[:, 0:1], gam[:, t:t + 1], pse[:, 1:2])
                nc.vector.tensor_mul(ab_t[:, 2:3], pse[:, 0:1], ab_t[:, 0:1])
                nc.vector.tensor_sub(ab_t[:, 1:2], bet[:, t:t + 1], ab_t[:, 2:3])
                if t % 2 == 0:
                    nc.scalar.activation(hs8[:, t * N:(t + 1) * N], xbs[t][:],
                                         Act.Identity,
                                         bias=ab_t[:, 1:2], scale=ab_t[:, 0:1])
                else:
                    nc.vector.tensor_scalar(hs8[:, t * N:(t + 1) * N],
                                            xbs[t][:],
                                            ab_t[:, 0:1], ab_t[:, 1:2],
                                            op0=Alu.mult, op1=Alu.add)
                junk_mm(4, last=(t == NT - 1))

            qks = [None] * NO_QK
            ppvs = [None] * NUM_HEADS
            prss = [None] * NUM_HEADS

            def emit_qk_half(ot, half):
                # fp8 DoubleRow: contract 512 channels as 2 pairs of 128
                if emit_qk_half.pq[ot] is None:
                    emit_qk_half.pq[ot] = pm.tile([128, N], f32, tag="acc",
                                                  name=f"pq{ot}")
                pq = emit_qk_half.pq[ot]
                for kp in range(2):
                    nc.tensor.matmul(
                        pq[:, half * 512:(half + 1) * 512],
                        wt8_3[:, 2 * kp:2 * kp + 2, ot * 128:(ot + 1) * 128],
                        hs8_3[:, 2 * kp:2 * kp + 2, half * 512:(half + 1) * 512],
                        start=(kp == 0), stop=(kp == 1), perf_mode=DR)
                if half == 1:
                    qk_t = pp.tile([128, N], bf16, tag=f"qk{ot}",
                                   name=f"qk{ot}")
                    nc.vector.tensor_scalar_add(qk_t[:], pq[:],
                                                bqk[:, ot:ot + 1])
                    qks[ot] = qk_t
            emit_qk_half.pq = [None] * NO_QK

            def emit_qk(ot):
                emit_qk_half(ot, 0)
                emit_qk_half(ot, 1)

            def emit_v(nt):
                pv_ = pm.tile([128, N], f32, tag="acc", name=f"pvv{nt}")
                for kp in range(2):
                    nc.tensor.matmul(
                        pv_[:, 0:512],
                        hs8_3[:, 2 * kp:2 * kp + 2, nt * 128:(nt + 1) * 128],
                        wt8_3[:, 2 * kp:2 * kp + 2, 2 * C:3 * C],
                        start=(kp == 0), stop=(kp == 1), perf_mode=DR)
                nc.vector.tensor_copy(v8[:, nt * C:(nt + 1) * C],
                                      pv_[:, 0:512])

            def emit_st(h, jt):
                qT = qks[h]
                kT = qks[NUM_HEADS + h]
                pst = pm.tile([128, N], f32, tag="ps", name=f"pst{h}_{jt}")
                for half in range(2):
                    nc.tensor.matmul(
                        pst[:, half * 512:(half + 1) * 512],
                        kT[:, jt * 128:(jt + 1) * 128],
                        qT[:, half * 512:(half + 1) * 512],
                        start=True, stop=True)
                nc.scalar.activation(pt8[h][:, jt * N:(jt + 1) * N], pst[:],
                                     Act.Exp, scale=SCALE, bias=shv[:, 0:1])

            def emit_rowsum_unit(h, half, p4):
                # DoubleRow ones-matmul: rowsum of a jt-pair of exp tiles
                if prss[h] is None:
                    prss[h] = pm.tile([128, N], f32, tag="acc",
                                      name=f"prs{h}")
                nc.tensor.matmul(
                    prss[h][0:1, half * 512:(half + 1) * 512],
                    ones8_3,
                    pt8_3[h][:, 2 * p4:2 * p4 + 2,
                             half * 512:(half + 1) * 512],
                    start=(p4 == 0), stop=(p4 == 3), perf_mode=DR)

            def emit_recip_bcast(h):
                nc.vector.reciprocal_approx_fast(rrs[h][:],
                                                 prss[h][0:1, :])
                nc.gpsimd.partition_broadcast(rbs[h][:], rrs[h][:])

            def emit_pv_unit(h, half, p4):
                if ppvs[h] is None:
                    ppvs[h] = pm.tile([128, N], f32, tag="acc", name=f"ppv{h}")
                nc.tensor.matmul(
                    ppvs[h][:, half * 512:(half + 1) * 512],
                    v8_3[:, 2 * p4:2 * p4 + 2, h * 128:(h + 1) * 128],
                    pt8_3[h][:, 2 * p4:2 * p4 + 2,
                             half * 512:(half + 1) * 512],
                    start=(p4 == 0), stop=(p4 == 3), perf_mode=DR)

            def emit_pv(h):
                for half in range(2):
                    for p4 in range(4):
                        emit_pv_unit(h, half, p4)

            def emit_norm(h, half=None):
                halves = range(2) if half is None else [half]
                for hf in halves:
                    sl = slice(h * N + hf * 512, h * N + (hf + 1) * 512)
                    psl = slice(hf * 512, (hf + 1) * 512)
                    nc.vector.tensor_mul(attn8[:, sl], ppvs[h][:, psl],
                                         rbs[h][:, psl])

            # ---- attention pipeline, interleaved so PE never waits on ACT
            emit_qk(0)
            emit_qk(4)
            emit_st(0, 0)
            emit_qk_half(1, 0); emit_st(0, 1)
            emit_qk_half(1, 1); emit_st(0, 2)
            emit_qk_half(5, 0); emit_st(0, 3)
            emit_qk_half(5, 1); emit_st(0, 4)
            emit_qk_half(2, 0); emit_st(0, 5)
            emit_qk_half(2, 1); emit_st(0, 6)
            emit_qk_half(6, 0); emit_st(0, 7)
            emit_qk_half(6, 1)
            emit_st(1, 0)
            emit_qk_half(3, 0); emit_st(1, 1)
            emit_qk_half(3, 1); emit_st(1, 2)
            emit_qk_half(7, 0); emit_st(1, 3)
            emit_qk_half(7, 1); emit_st(1, 4)
            emit_v(0); emit_st(1, 5)
            emit_v(1); emit_st(1, 6)
            emit_v(2); emit_st(1, 7)
            emit_v(3)
            emit_st(2, 0)
            emit_v(4); emit_st(2, 1)
            emit_v(5); emit_st(2, 2)
            emit_v(6); emit_st(2, 3)
            emit_v(7); emit_st(2, 4)
            emit_rowsum_unit(0, 0, 0); emit_st(2, 5)
            emit_rowsum_unit(0, 0, 1); emit_st(2, 6)
            emit_rowsum_unit(0, 0, 2); emit_st(2, 7)
            emit_rowsum_unit(0, 0, 3)
            emit_st(3, 0)
            emit_rowsum_unit(0, 1, 0); emit_rowsum_unit(0, 1, 1)
            emit_st(3, 1)
            emit_rowsum_unit(0, 1, 2); emit_rowsum_unit(0, 1, 3)
            emit_recip_bcast(0)
            emit_st(3, 2)
            emit_rowsum_unit(1, 0, 0); emit_rowsum_unit(1, 0, 1)
            emit_st(3, 3)
            emit_rowsum_unit(1, 0, 2); emit_rowsum_unit(1, 0, 3)
            emit_st(3, 4)
            emit_rowsum_unit(1, 1, 0); emit_rowsum_unit(1, 1, 1)
            emit_st(3, 5)
            emit_rowsum_unit(1, 1, 2); emit_rowsum_unit(1, 1, 3)
            emit_recip_bcast(1)
            emit_st(3, 6)
            emit_pv_unit(0, 0, 0); emit_pv_unit(0, 0, 1)
            emit_st(3, 7)
            emit_pv_unit(0, 0, 2); emit_pv_unit(0, 0, 3)
            emit_norm(0, 0)
            emit_pv_unit(0, 1, 0); emit_pv_unit(0, 1, 1)
            emit_pv_unit(0, 1, 2); emit_pv_unit(0, 1, 3)
            emit_norm(0, 1)
            for hf in range(2):
                for p4 in range(4):
                    emit_pv_unit(1, hf, p4)
                emit_norm(1, hf)
            for half in range(2):
                for p4 in range(4):
                    emit_rowsum_unit(2, half, p4)
            emit_recip_bcast(2)
            for hf in range(2):
                for p4 in range(4):
                    emit_pv_unit(2, hf, p4)
                emit_norm(2, hf)
            for half in range(2):
                for p4 in range(4):
                    emit_rowsum_unit(3, half, p4)
            emit_recip_bcast(3)
            for hf in range(2):
                for p4 in range(4):
                    emit_pv_unit(3, hf, p4)
                emit_norm(3, hf)

            # ---- proj + bias + residual (fp8 DoubleRow over head pairs);
            # bias+residual and the store are pipelined per half tile
            for ot in range(NT):
                ppr = pm.tile([128, N], f32, tag="acc", name=f"ppr{ot}")
                o_t = p_out.tile([128, N], f32, tag="out", name=f"o{ot}")
                for half in range(2):
                    sl = slice(half * 512, (half + 1) * 512)
                    for hp in range(2):
                        nc.tensor.matmul(
                            ppr[:, sl],
                            pwt8_3[:, 2 * hp:2 * hp + 2,
                                   ot * 128:(ot + 1) * 128],
                            attn8_3[:, 2 * hp:2 * hp + 2, sl],
                            start=(hp == 0), stop=(hp == 1), perf_mode=DR)
                    nc.vector.scalar_tensor_tensor(o_t[:, sl], ppr[:, sl],
                                                   beff[:, ot:ot + 1],
                                                   xbs[ot][:, sl],
                                                   op0=Alu.add, op1=Alu.add)
                eng = nc.scalar if ot % 2 == 1 else nc.sync
                eng.dma_start(d_out[ot * 128:(ot + 1) * 128, :], o_t[:])

    nc.compile()
    return nc


def make_in_maps(x, norm_w, norm_b, qkv_w, qkv_b, proj_w, proj_b):
    import ml_dtypes
    f8 = ml_dtypes.float8_e4m3

    x = np.asarray(x, dtype=np.float32)
    qkv_w = np.asarray(qkv_w, dtype=np.float32)
    qkv_b = np.asarray(qkv_b, dtype=np.float32)
    proj_w = np.asarray(proj_w, dtype=np.float32)
    proj_b = np.asarray(proj_b, dtype=np.float32)

    # DoubleRow k-subtile layout: wt8[c, k*1536+o] = qkv_w[o, c+128k]
    wt8 = np.ascontiguousarray(
        qkv_w.T.reshape(4, 128, 3 * C).transpose(1, 0, 2).reshape(128, -1)
    ).astype(f8)
    pwt8 = np.ascontiguousarray(
        proj_w.T.reshape(4, 128, C).transpose(1, 0, 2).reshape(128, -1)
    ).astype(f8)

    b_eff = (proj_b + proj_w @ qkv_b[2 * C:3 * C]).astype(np.float32)
    bias_qk = np.ascontiguousarray(qkv_b[:2 * C])

    p = np.arange(128)
    sel = (p[:, None] // GROUP_CH == np.arange(8)[None, :]).astype(np.float32)
    selT = np.ascontiguousarray(sel.T)

    xs = x.reshape(B, C, N)
    cvec = np.zeros((128, 28), np.float32)
    cvec[:, 0:4] = np.asarray(norm_w, np.float32).reshape(4, 128).T
    cvec[:, 4:8] = np.asarray(norm_b, np.float32).reshape(4, 128).T
    cvec[:, 8:16] = bias_qk.reshape(8, 128).T
    cvec[:, 16:20] = b_eff.reshape(4, 128).T
    cvec[:, 20:28] = sel
    common = {
        "qkv_wt8": wt8, "proj_wt8": pwt8, "cvec": cvec, "selT": selT,
        "ones8": np.ones((128, 32), np.float32).astype(f8),
    }
    return [dict(common,
                 xb=np.ascontiguousarray(xs[i]).astype(ml_dtypes.bfloat16))
            for i in range(B)]


def run(inputs, trace=False, tmpdir=None):
    from concourse.bass_utils import run_bass_kernel_spmd
    nc = build_bass()
    in_maps = make_in_maps(**inputs)
    res = run_bass_kernel_spmd(nc, in_maps, core_ids=list(range(N_CORES)),
                               trace=trace, tmpdir=tmpdir)
    out = np.stack([res.results[i]["out"] for i in range(N_CORES)])
    return out.reshape(B, C, H, W).astype(np.float32), res


def kernel(**inputs):
    out, _ = run(inputs, trace=False)
    return out
